# revision 25
# baseline (speedup 1.0000x reference)
"""MoE gate (router) kernel for Trainium2, 8 NeuronCores.

Computes, for hidden_states [4, 8192, 4096] fp32 and weight [64, 4096] fp32:
    logits = x @ W.T        # [T=32768, 64]
    scores = softmax(logits)
    topk_weight, topk_idx = top_k(scores, 2)
returns (topk_idx int32 [T, 2], topk_weight fp32 [T, 2]).

Sharding: tokens split evenly across 8 cores (4096 tokens/core); the small
gate weight is replicated. No collectives needed.

Per-core dataflow (hi/lo bf16 split preserves top-2 ordering; min top2/top3
logit gap in this regime is ~2e-5, so plain bf16 would mis-rank rows):
  - DMA x naturally as [128 tok, 2048 h] half-tiles.
  - PE transpose-mode flips 128x128 blocks to get h on partitions (PSUM),
    ACT cast-copies PSUM->SBUF as bf16 hi, DVE subtract produces bf16 lo.
  - PE matmul accumulates a [128, 512] PSUM tile per 512-token group over
    32 h-chunks with a PACKED stationary [Wh_c | Wl_c] (128 wide, full PE
    array): rows 0:64 accumulate xh@Wh (+ xl@Wh from a second 64-wide
    matmul into the sub-range), rows 64:128 accumulate xh@Wl. Two moving
    streams per chunk instead of three.
  - Epilogue per group: ACT copy PSUM->SBUF, PE transpose [128,128] blocks
    to token-major [128 t, 128 e2], ACT copy hi half to SBUF, DVE add gives
    logits [128, 64]; VectorE max/max_index yields top-8; ScalarE Exp with
    accum_out gives the softmax denominator; reciprocal+scale produce the
    two weights.
  - Results staged in SBUF [128, 2*n_tok_tiles], single DMA out; host
    unshuffles the [tile, partition] interleave.
"""

import sys

for _p in ("/opt/trn_rl_repo", "/root/.axon_site/_ro/trn_rl_repo"):
    if _p not in sys.path:
        sys.path.append(_p)

import numpy as np

import concourse.bass as bass
import concourse.bacc as bacc
import concourse.mybir as mybir
from concourse.tile import TileContext
from concourse.bass_utils import run_bass_kernel_spmd

N_CORES = 8
H = 4096
E = 64
P = 128
N_CHUNK = H // P  # 32 contraction chunks
TOK_GRP = 512  # tokens per PSUM logits accumulation group
F32 = mybir.dt.float32
BF16 = mybir.dt.bfloat16
I32 = mybir.dt.int32
U32 = mybir.dt.uint32


def build_nc(t_core: int) -> bass.Bass:
    """Build the per-core Bass module for t_core tokens."""
    assert t_core % TOK_GRP == 0
    n_grp = t_core // TOK_GRP
    n_tiles = t_core // P  # token tiles of 128

    # Bacc (not raw Bass): its compile() pipeline legalizes semaphore waits
    # (move_matmul_waits_to_ldweights, event-sem conversion) for the 1-wait
    # EVENTS slot walrus enforces per instruction.
    nc = bacc.Bacc(trn_type="TRN2")
    x_d = nc.dram_tensor("x", [t_core, H], F32, kind="ExternalInput")
    # whl layout: whl[p, c*128 + j] = Wh[j, 128c+p] for j<64, Wl[j-64, ...]
    # for j>=64 (host-prepared). whv = Wh only, for the xl correction term.
    whl_d = nc.dram_tensor("whl", [P, N_CHUNK * 2 * E], BF16, kind="ExternalInput")
    idb_d = nc.dram_tensor("ident_b", [8, 8], BF16, kind="ExternalInput")
    id_d = nc.dram_tensor("ident", [P, P], F32, kind="ExternalInput")
    ow_d = nc.dram_tensor("out_w", [P, 2 * n_tiles], F32, kind="ExternalOutput")
    oi_d = nc.dram_tensor("out_i", [P, 2 * n_tiles], I32, kind="ExternalOutput")

    with TileContext(nc) as tc:
        with (
            tc.tile_pool(name="const", bufs=1) as cpool,
            tc.tile_pool(name="xnat", bufs=16) as xpool,
            tc.tile_pool(name="xt", bufs=8) as xtpool,
            tc.tile_pool(name="ptr", bufs=3, space="PSUM") as pt_pool,
            tc.tile_pool(name="plog", bufs=2, space="PSUM") as pl_pool,
            tc.tile_pool(name="plt", bufs=1, space="PSUM") as plt_pool,
            tc.tile_pool(name="sac", bufs=1, space="PSUM") as sac_pool,
            tc.tile_pool(name="sacb", bufs=1, space="PSUM") as sacb_pool,
            tc.tile_pool(name="small", bufs=4) as spool,
            tc.tile_pool(name="outs", bufs=1) as opool,
        ):
            # Small consts first so the PE absorbers/warmup unblock early,
            # then the weights, then the x stream follows on the same ring.
            ident = cpool.tile([P, P], F32)
            nc.sync.dma_start(ident[:], id_d[:])
            idb_sb = cpool.tile([8, 8], BF16)
            nc.sync.dma_start(idb_sb[:], idb_d[:])
            whl_sb = cpool.tile([P, N_CHUNK * 2 * E], BF16)
            nc.sync.dma_start(whl_sb[:], whl_d[:])
            ow_sb = opool.tile([P, 2 * n_tiles], F32)
            oi_sb = opool.tile([P, 2 * n_tiles], I32)

            # Every TPB instruction has ONE sem-wait slot, and walrus cannot
            # split multi-waits for the fused fp32 matmul. So each DMA'd tile
            # gets a tiny sacrificial 8x8 PE transpose ("absorber") that
            # carries the DMA wait; real PE work then sees the tick as
            # observed and needs at most one other wait. Absorber outputs go
            # to distinct columns of one never-recycled PSUM bank (no WAW).
            # cols 0..239: HAM-warmup scratch; cols 240+: absorber outputs
            sac = sac_pool.tile([4, 240 + 4 * (2 + 8 * n_grp)], F32)
            n_sac = 0

            def absorb(src_ap):
                nonlocal n_sac
                nc.tensor.transpose(
                    sac[:, 240 + 4 * n_sac : 244 + 4 * n_sac], src_ap, ident[0:4, 0:4]
                )
                n_sac += 1

            absorb(ident[0:4, 0:4])
            # bf16 absorbers need a bf16 PSUM target (transpose output
            # dtype must match input) and a bf16 identity rhs.
            sac_bf = sacb_pool.tile([8, 24], BF16)
            nc.tensor.transpose(sac_bf[:, 0:8], idb_sb[:], idb_sb[:])
            # HAM warmup: ~6us of back-to-back matmuls while whl and the
            # first x tiles stream in, so real work starts at 2.4 GHz
            # instead of paying ~20us of half-clock ramp. The moving operand
            # is a bf16 BITCAST view of ident (values irrelevant, arrives
            # ~10us before whl), so the warmup gates on nothing but the two
            # small const DMAs. They reuse the sac bank (serial same-engine
            # WAW, no sems needed); later absorber writes order behind them.
            ident_bf = ident.bitcast(BF16)
            for _ in range(50):
                nc.tensor.matmul(
                    sac[:, 0:240], idb_sb[0:8, 0:4], ident_bf[0:8, 0:240],
                    start=True, stop=True, skip_group_check=True,
                )
            # whl's DMA-wait absorber sits AFTER the warmup so it cannot
            # delay the ramp; mm1 of chunk 0 is the first real consumer.
            nc.tensor.transpose(sac_bf[:, 8:16], whl_sb[0:8, 0:8], idb_sb[:])

            HH = H // 2

            # Software pipeline, carried ACROSS group boundaries: the matmul
            # pair for chunk (g, c) is emitted after the transposes of chunk
            # c+DELAY (possibly in group g+1), so the serial ACT-hi -> DVE-lo
            # chain always finishes before PE program order reaches mm2, and
            # the old per-group pend flush (6 back-to-back bf16 matmuls, each
            # paying the stationary-file serialization) disappears. Each
            # group's epilogue is emitted right after its stop=True matmul
            # pops from the queue.
            DELAY = 5
            pend = []  # (logits_ps, g, c, xh_sb, xl_sb)

            def emit_epilogue(g, logits_ps):
                # psum [128, 512]: rows 0:64 = xh@Wh + xl@Wh, rows 64:128 =
                # xh@Wl. Transpose the lo/hi expert halves of each 128-token
                # block into contiguous regions of ONE l_ps bank (lo halves
                # at cols 0:256, hi halves at 256:512), so a single ACT copy
                # + a single DVE add combine them for the whole group.
                lt_sb = spool.tile([P, TOK_GRP], F32, tag="lt", name=f"lt_{g}")
                nc.scalar.copy(lt_sb[:], logits_ps[:])
                # All four token-blocks transpose into ONE l_ps bank (disjoint
                # column ranges), so the per-tb ACT copies / DVE adds have no
                # WAR serialization between blocks and pipeline freely.
                l_ps = plt_pool.tile([P, TOK_GRP], F32, tag="lps", name=f"lps_{g}")
                for tb in range(4):
                    nc.tensor.transpose(
                        l_ps[:, bass.ts(tb, P)], lt_sb[:, bass.ts(tb, P)], ident[:]
                    )
                l_sb = spool.tile([P, 4 * E], F32, tag="lsb", name=f"lsb_{g}")
                for tb in range(4):
                    hi_sb = spool.tile(
                        [P, E], F32, tag="hi", name=f"hi_{g}_{tb}"
                    )
                    # ACT (not DVE) so the l_ps bank WAR release stays on the
                    # ACT sem PE already tracks — keeps PE waits ≤1 per inst.
                    nc.scalar.copy(hi_sb[:], l_ps[:, tb * P + E : (tb + 1) * P])
                    nc.vector.tensor_tensor(
                        l_sb[:, bass.ts(tb, E)],
                        l_ps[:, tb * P : tb * P + E],
                        hi_sb[:],
                        mybir.AluOpType.add,
                    )
                for tb in range(4):
                    col = g * 4 + tb
                    lv = l_sb[:, bass.ts(tb, E)]
                    mx = spool.tile([P, 8], F32, tag="mx", name=f"mx_{col}")
                    nc.vector.max(mx[:], lv)
                    mi = spool.tile([P, 8], U32, tag="mi", name=f"mi_{col}")
                    nc.vector.max_index(mi[:], mx[:], lv)
                    ex = spool.tile([P, E], F32, tag="ex", name=f"ex_{col}")
                    ssum = spool.tile([P, 1], F32, tag="ss", name=f"ss_{col}")
                    nc.scalar.activation(
                        ex[:],
                        lv,
                        mybir.ActivationFunctionType.Exp,
                        accum_out=ssum[:],
                    )
                    e2 = spool.tile([P, 2], F32, tag="e2", name=f"e2_{col}")
                    nc.scalar.activation(
                        e2[:], mx[:, 0:2], mybir.ActivationFunctionType.Exp
                    )
                    rec = spool.tile([P, 1], F32, tag="rc", name=f"rc_{col}")
                    nc.vector.reciprocal(rec[:], ssum[:])
                    nc.vector.tensor_scalar(
                        ow_sb[:, bass.ts(col, 2)],
                        e2[:],
                        rec[:],
                        None,
                        op0=mybir.AluOpType.mult,
                    )
                    nc.vector.tensor_copy(oi_sb[:, bass.ts(col, 2)], mi[:, 0:2])

            def emit_mms(logits_ps, g, c, xh_sb, xl_sb):
                nc.tensor.matmul(
                    logits_ps[:], whl_sb[:, bass.ts(c, 2 * E)], xh_sb[:],
                    start=(c == 0), stop=False, skip_group_check=True,
                )
                nc.tensor.matmul(
                    logits_ps[0:E, :],
                    whl_sb[:, c * 2 * E : c * 2 * E + E],
                    xl_sb[:],
                    start=False, stop=(c == N_CHUNK - 1),
                    skip_group_check=True,
                )
                if c == N_CHUNK - 1:
                    emit_epilogue(g, logits_ps)

            for g in range(n_grp):
                # Two half-H tiles per token block: halves the DMA granularity
                # (1 MB each) so the next group's first-half loads can start a
                # half-group earlier — removes the ~6us group-boundary stalls.
                # Emit hh=0 loads for all four blocks first: the first 16
                # chunks only need the hh=0 halves, so they arrive sooner.
                xnats = [[None, None] for _ in range(4)]
                for hh in range(2):
                    for tb in range(4):
                        row = bass.ts(g * 4 + tb, P)
                        xn = xpool.tile(
                            [P, HH], F32, tag="xn", name=f"xn_{g}_{tb}_{hh}"
                        )
                        nc.sync.dma_start(
                            xn[:], x_d[row, hh * HH : (hh + 1) * HH]
                        )
                        absorb(xn[0:4, 0:4])
                        xnats[tb][hh] = xn

                logits_ps = pl_pool.tile([P, TOK_GRP], F32, tag="lg", name=f"lg_{g}")
                for c in range(N_CHUNK):
                    xt_ps = pt_pool.tile(
                        [P, TOK_GRP], F32, tag="xtp", name=f"xtp_{g}_{c}"
                    )
                    for tb in range(4):
                        nc.tensor.transpose(
                            xt_ps[:, bass.ts(tb, P)],
                            xnats[tb][c // 16][:, bass.ts(c % 16, P)],
                            ident[:],
                        )
                    xh_sb = xtpool.tile(
                        [P, TOK_GRP], BF16, tag="xh", name=f"xh_{g}_{c}"
                    )
                    xl_sb = xtpool.tile(
                        [P, TOK_GRP], BF16, tag="xl", name=f"xl_{g}_{c}"
                    )
                    nc.scalar.copy(xh_sb[:], xt_ps[:])
                    nc.vector.tensor_tensor(
                        xl_sb[:], xt_ps[:], xh_sb[:],
                        mybir.AluOpType.subtract,
                    )
                    pend.append((logits_ps, g, c, xh_sb, xl_sb))
                    if len(pend) > DELAY:
                        emit_mms(*pend.pop(0))

            while pend:
                emit_mms(*pend.pop(0))

            nc.sync.dma_start(ow_d[:], ow_sb[:])
            nc.sync.dma_start(oi_d[:], oi_sb[:])
    nc.compile()
    return nc


def _prep_inputs(hidden_states, weight, t_core):
    import ml_dtypes

    x = np.ascontiguousarray(
        np.asarray(hidden_states, dtype=np.float32).reshape(-1, H)
    )
    w = np.asarray(weight, dtype=np.float32)
    wh = w.astype(ml_dtypes.bfloat16)
    wl = (w - wh.astype(np.float32)).astype(ml_dtypes.bfloat16)
    whl = np.zeros((P, N_CHUNK * 2 * E), dtype=ml_dtypes.bfloat16)
    for c in range(N_CHUNK):
        whl[:, c * 2 * E : c * 2 * E + E] = wh[:, c * P : (c + 1) * P].T
        whl[:, c * 2 * E + E : (c + 1) * 2 * E] = wl[:, c * P : (c + 1) * P].T
    consts = {
        "whl": whl,
        "ident": np.eye(P, dtype=np.float32),
        "ident_b": np.eye(8, dtype=ml_dtypes.bfloat16),
    }
    n = x.shape[0] // t_core
    in_maps = [
        {"x": np.ascontiguousarray(x[i * t_core : (i + 1) * t_core]), **consts}
        for i in range(n)
    ]
    return in_maps


def _unshuffle(res_list, t_core):
    n_tiles = t_core // P
    t_full = t_core * len(res_list)
    idx = np.empty((t_full, 2), np.int32)
    wts = np.empty((t_full, 2), np.float32)
    for i, r in enumerate(res_list):
        ow = r["out_w"].reshape(P, n_tiles, 2).transpose(1, 0, 2).reshape(t_core, 2)
        oi = r["out_i"].reshape(P, n_tiles, 2).transpose(1, 0, 2).reshape(t_core, 2)
        wts[i * t_core : (i + 1) * t_core] = ow
        idx[i * t_core : (i + 1) * t_core] = oi
    return idx, wts


_NC_CACHE: dict = {}


def run(hidden_states, weight, trace=False, **kw):
    t_full = int(np.prod(np.asarray(hidden_states).shape[:-1]))
    t_core = t_full // N_CORES
    key = t_core
    if key not in _NC_CACHE:
        _NC_CACHE[key] = build_nc(t_core)
    nc = _NC_CACHE[key]
    in_maps = _prep_inputs(hidden_states, weight, t_core)
    br = run_bass_kernel_spmd(
        nc, in_maps, core_ids=list(range(len(in_maps))), trace=trace, **kw
    )
    idx, wts = _unshuffle(br.results, t_core)
    return idx, wts, br


def kernel(hidden_states, weight):
    idx, wts, _ = run(hidden_states, weight)
    return idx, wts


# revision 27
# speedup vs baseline: 1.0033x; 1.0033x over previous
"""MoE gate (router) kernel for Trainium2, 8 NeuronCores.

Computes, for hidden_states [4, 8192, 4096] fp32 and weight [64, 4096] fp32:
    logits = x @ W.T        # [T=32768, 64]
    scores = softmax(logits)
    topk_weight, topk_idx = top_k(scores, 2)
returns (topk_idx int32 [T, 2], topk_weight fp32 [T, 2]).

Sharding: tokens split evenly across 8 cores (4096 tokens/core); the small
gate weight is replicated. No collectives needed.

Per-core dataflow (hi/lo bf16 split preserves top-2 ordering; min top2/top3
logit gap in this regime is ~2e-5, so plain bf16 would mis-rank rows):
  - DMA x naturally as [128 tok, 2048 h] half-tiles.
  - PE transpose-mode flips 128x128 blocks to get h on partitions (PSUM),
    ACT cast-copies PSUM->SBUF as bf16 hi, DVE subtract produces bf16 lo.
  - PE matmul accumulates a [128, 512] PSUM tile per 512-token group over
    32 h-chunks with a PACKED stationary [Wh_c | Wl_c] (128 wide, full PE
    array): rows 0:64 accumulate xh@Wh (+ xl@Wh from a second 64-wide
    matmul into the sub-range), rows 64:128 accumulate xh@Wl. Two moving
    streams per chunk instead of three.
  - Epilogue per group: ACT copy PSUM->SBUF, PE transpose [128,128] blocks
    to token-major [128 t, 128 e2], ACT copy hi half to SBUF, DVE add gives
    logits [128, 64]; VectorE max/max_index yields top-8; ScalarE Exp with
    accum_out gives the softmax denominator; reciprocal+scale produce the
    two weights.
  - Results staged in SBUF [128, 2*n_tok_tiles], single DMA out; host
    unshuffles the [tile, partition] interleave.
"""

import sys

for _p in ("/opt/trn_rl_repo", "/root/.axon_site/_ro/trn_rl_repo"):
    if _p not in sys.path:
        sys.path.append(_p)

import numpy as np

import concourse.bass as bass
import concourse.bacc as bacc
import concourse.mybir as mybir
from concourse.tile import TileContext
from concourse.bass_utils import run_bass_kernel_spmd

N_CORES = 8
H = 4096
E = 64
P = 128
N_CHUNK = H // P  # 32 contraction chunks
TOK_GRP = 512  # tokens per PSUM logits accumulation group
F32 = mybir.dt.float32
BF16 = mybir.dt.bfloat16
I32 = mybir.dt.int32
U32 = mybir.dt.uint32


def build_nc(t_core: int) -> bass.Bass:
    """Build the per-core Bass module for t_core tokens."""
    assert t_core % TOK_GRP == 0
    n_grp = t_core // TOK_GRP
    n_tiles = t_core // P  # token tiles of 128

    # Bacc (not raw Bass): its compile() pipeline legalizes semaphore waits
    # (move_matmul_waits_to_ldweights, event-sem conversion) for the 1-wait
    # EVENTS slot walrus enforces per instruction.
    nc = bacc.Bacc(trn_type="TRN2")
    x_d = nc.dram_tensor("x", [t_core, H], F32, kind="ExternalInput")
    # whl layout: whl[p, c*128 + j] = Wh[j, 128c+p] for j<64, Wl[j-64, ...]
    # for j>=64 (host-prepared). The xl matmul reads the Wh half directly.
    whl_d = nc.dram_tensor("whl", [P, N_CHUNK * 2 * E], BF16, kind="ExternalInput")
    idb_d = nc.dram_tensor("ident_b", [8, 8], BF16, kind="ExternalInput")
    id_d = nc.dram_tensor("ident", [P, P], F32, kind="ExternalInput")
    ow_d = nc.dram_tensor("out_w", [P, 2 * n_tiles], F32, kind="ExternalOutput")
    oi_d = nc.dram_tensor("out_i", [P, 2 * n_tiles], I32, kind="ExternalOutput")

    with TileContext(nc) as tc:
        with (
            tc.tile_pool(name="const", bufs=1) as cpool,
            tc.tile_pool(name="xnat", bufs=16) as xpool,
            tc.tile_pool(name="xt", bufs=8) as xtpool,
            tc.tile_pool(name="ptr", bufs=3, space="PSUM") as pt_pool,
            tc.tile_pool(name="plog", bufs=2, space="PSUM") as pl_pool,
            tc.tile_pool(name="plt", bufs=1, space="PSUM") as plt_pool,
            tc.tile_pool(name="sac", bufs=1, space="PSUM") as sac_pool,
            tc.tile_pool(name="sacb", bufs=1, space="PSUM") as sacb_pool,
            tc.tile_pool(name="small", bufs=4) as spool,
            tc.tile_pool(name="outs", bufs=1) as opool,
        ):
            # Small consts first so the PE absorbers/warmup unblock early,
            # then the weights, then the x stream follows on the same ring.
            ident = cpool.tile([P, P], F32)
            nc.sync.dma_start(ident[:], id_d[:])
            idb_sb = cpool.tile([8, 8], BF16)
            nc.sync.dma_start(idb_sb[:], idb_d[:])
            whl_sb = cpool.tile([P, N_CHUNK * 2 * E], BF16)
            nc.sync.dma_start(whl_sb[:], whl_d[:])
            ow_sb = opool.tile([P, 2 * n_tiles], F32)
            oi_sb = opool.tile([P, 2 * n_tiles], I32)

            # Every TPB instruction has ONE sem-wait slot, and walrus cannot
            # split multi-waits for the fused fp32 matmul. So each DMA'd tile
            # gets a tiny sacrificial 8x8 PE transpose ("absorber") that
            # carries the DMA wait; real PE work then sees the tick as
            # observed and needs at most one other wait. Absorber outputs go
            # to distinct columns of one never-recycled PSUM bank (no WAW).
            # cols 0..239: HAM-warmup scratch; cols 240+: absorber outputs
            sac = sac_pool.tile([4, 240 + 4 * (2 + 8 * n_grp)], F32)
            n_sac = 0

            def absorb(src_ap):
                nonlocal n_sac
                nc.tensor.transpose(
                    sac[:, 240 + 4 * n_sac : 244 + 4 * n_sac], src_ap, ident[0:4, 0:4]
                )
                n_sac += 1

            absorb(ident[0:4, 0:4])
            # bf16 absorbers need a bf16 PSUM target (transpose output
            # dtype must match input) and a bf16 identity rhs.
            sac_bf = sacb_pool.tile([8, 24], BF16)
            nc.tensor.transpose(sac_bf[:, 0:8], idb_sb[:], idb_sb[:])
            # HAM warmup: ~6us of back-to-back matmuls while whl and the
            # first x tiles stream in, so real work starts at 2.4 GHz
            # instead of paying ~20us of half-clock ramp. The moving operand
            # is a bf16 BITCAST view of ident (values irrelevant, arrives
            # ~10us before whl), so the warmup gates on nothing but the two
            # small const DMAs. They reuse the sac bank (serial same-engine
            # WAW, no sems needed); later absorber writes order behind them.
            ident_bf = ident.bitcast(BF16)
            for _ in range(50):
                nc.tensor.matmul(
                    sac[:, 0:240], idb_sb[0:8, 0:4], ident_bf[0:8, 0:240],
                    start=True, stop=True, skip_group_check=True,
                )
            # whl's DMA-wait absorber sits AFTER the warmup so it cannot
            # delay the ramp; mm1 of chunk 0 is the first real consumer.
            nc.tensor.transpose(sac_bf[:, 8:16], whl_sb[0:8, 0:8], idb_sb[:])

            HH = H // 2

            # Software pipeline, carried ACROSS group boundaries: the matmul
            # pair for chunk (g, c) is emitted after the transposes of chunk
            # c+DELAY (possibly in group g+1), so the serial ACT-hi -> DVE-lo
            # chain always finishes before PE program order reaches mm2, and
            # the old per-group pend flush (6 back-to-back bf16 matmuls, each
            # paying the stationary-file serialization) disappears. Each
            # group's epilogue is emitted right after its stop=True matmul
            # pops from the queue.
            DELAY = 5
            pend = []  # (logits_ps, g, c, xh_sb, xl_sb)

            def emit_epilogue(g, logits_ps):
                # psum [128, 512]: rows 0:64 = xh@Wh + xl@Wh, rows 64:128 =
                # xh@Wl. Transpose each 128-token block token-major into one
                # 3D l_ps bank [P, 4, 128]; the lo/hi expert halves are then
                # combined for the WHOLE group with a single strided ACT copy
                # + a single DVE add, and e2/recip batch across blocks too.
                lt_sb = spool.tile([P, TOK_GRP], F32, tag="lt", name=f"lt_{g}")
                nc.scalar.copy(lt_sb[:], logits_ps[:])
                l_ps = plt_pool.tile([P, 4, P], F32, tag="lps", name=f"lps_{g}")
                for tb in range(4):
                    nc.tensor.transpose(
                        l_ps[:, tb, :], lt_sb[:, bass.ts(tb, P)], ident[:]
                    )
                hi_sb = spool.tile([P, 4, E], F32, tag="hi", name=f"hi_{g}")
                # ACT (not DVE) so the l_ps bank WAR release stays on the
                # ACT sem PE already tracks — keeps PE waits ≤1 per inst.
                nc.scalar.copy(hi_sb[:], l_ps[:, :, E : 2 * E])
                l_sb = spool.tile([P, 4, E], F32, tag="lsb", name=f"lsb_{g}")
                nc.vector.tensor_tensor(
                    l_sb[:], l_ps[:, :, 0:E], hi_sb[:], mybir.AluOpType.add
                )
                mx = spool.tile([P, 4, 8], F32, tag="mx", name=f"mx_{g}")
                mi = spool.tile([P, 4, 8], U32, tag="mi", name=f"mi_{g}")
                ssum = spool.tile([P, 4], F32, tag="ss", name=f"ss_{g}")
                for tb in range(4):
                    col = g * 4 + tb
                    nc.vector.max(mx[:, tb, :], l_sb[:, tb, :])
                    nc.vector.max_index(mi[:, tb, :], mx[:, tb, :], l_sb[:, tb, :])
                    ex = spool.tile([P, E], F32, tag="ex", name=f"ex_{col}")
                    nc.scalar.activation(
                        ex[:],
                        l_sb[:, tb, :],
                        mybir.ActivationFunctionType.Exp,
                        accum_out=ssum[:, tb : tb + 1],
                    )
                e2 = spool.tile([P, 4, 2], F32, tag="e2", name=f"e2_{g}")
                nc.scalar.activation(
                    e2[:], mx[:, :, 0:2], mybir.ActivationFunctionType.Exp
                )
                rec = spool.tile([P, 4], F32, tag="rc", name=f"rc_{g}")
                nc.vector.reciprocal(rec[:], ssum[:])
                for tb in range(4):
                    col = g * 4 + tb
                    nc.vector.tensor_scalar(
                        ow_sb[:, bass.ts(col, 2)],
                        e2[:, tb, :],
                        rec[:, tb : tb + 1],
                        None,
                        op0=mybir.AluOpType.mult,
                    )
                    nc.vector.tensor_copy(
                        oi_sb[:, bass.ts(col, 2)], mi[:, tb, 0:2]
                    )

            def emit_mms(logits_ps, g, c, xh_sb, xl_sb):
                nc.tensor.matmul(
                    logits_ps[:], whl_sb[:, bass.ts(c, 2 * E)], xh_sb[:],
                    start=(c == 0), stop=False, skip_group_check=True,
                )
                nc.tensor.matmul(
                    logits_ps[0:E, :],
                    whl_sb[:, c * 2 * E : c * 2 * E + E],
                    xl_sb[:],
                    start=False, stop=(c == N_CHUNK - 1),
                    skip_group_check=True,
                )
                if c == N_CHUNK - 1:
                    emit_epilogue(g, logits_ps)

            for g in range(n_grp):
                # Two half-H tiles per token block: halves the DMA granularity
                # (1 MB each) so the next group's first-half loads can start a
                # half-group earlier — removes the ~6us group-boundary stalls.
                # Emit hh=0 loads for all four blocks first: the first 16
                # chunks only need the hh=0 halves, so they arrive sooner.
                xnats = [[None, None] for _ in range(4)]
                for hh in range(2):
                    for tb in range(4):
                        row = bass.ts(g * 4 + tb, P)
                        xn = xpool.tile(
                            [P, HH], F32, tag="xn", name=f"xn_{g}_{tb}_{hh}"
                        )
                        nc.sync.dma_start(
                            xn[:], x_d[row, hh * HH : (hh + 1) * HH]
                        )
                        absorb(xn[0:4, 0:4])
                        xnats[tb][hh] = xn

                logits_ps = pl_pool.tile([P, TOK_GRP], F32, tag="lg", name=f"lg_{g}")
                for c in range(N_CHUNK):
                    xt_ps = pt_pool.tile(
                        [P, TOK_GRP], F32, tag="xtp", name=f"xtp_{g}_{c}"
                    )
                    for tb in range(4):
                        nc.tensor.transpose(
                            xt_ps[:, bass.ts(tb, P)],
                            xnats[tb][c // 16][:, bass.ts(c % 16, P)],
                            ident[:],
                        )
                    xh_sb = xtpool.tile(
                        [P, TOK_GRP], BF16, tag="xh", name=f"xh_{g}_{c}"
                    )
                    xl_sb = xtpool.tile(
                        [P, TOK_GRP], BF16, tag="xl", name=f"xl_{g}_{c}"
                    )
                    nc.scalar.copy(xh_sb[:], xt_ps[:])
                    nc.vector.tensor_tensor(
                        xl_sb[:], xt_ps[:], xh_sb[:],
                        mybir.AluOpType.subtract,
                    )
                    pend.append((logits_ps, g, c, xh_sb, xl_sb))
                    if len(pend) > DELAY:
                        emit_mms(*pend.pop(0))

            while pend:
                emit_mms(*pend.pop(0))

            nc.sync.dma_start(ow_d[:], ow_sb[:])
            nc.sync.dma_start(oi_d[:], oi_sb[:])
    nc.compile()
    return nc


def _prep_inputs(hidden_states, weight, t_core):
    import ml_dtypes

    x = np.ascontiguousarray(
        np.asarray(hidden_states, dtype=np.float32).reshape(-1, H)
    )
    w = np.asarray(weight, dtype=np.float32)
    wh = w.astype(ml_dtypes.bfloat16)
    wl = (w - wh.astype(np.float32)).astype(ml_dtypes.bfloat16)
    whl = np.zeros((P, N_CHUNK * 2 * E), dtype=ml_dtypes.bfloat16)
    for c in range(N_CHUNK):
        whl[:, c * 2 * E : c * 2 * E + E] = wh[:, c * P : (c + 1) * P].T
        whl[:, c * 2 * E + E : (c + 1) * 2 * E] = wl[:, c * P : (c + 1) * P].T
    consts = {
        "whl": whl,
        "ident": np.eye(P, dtype=np.float32),
        "ident_b": np.eye(8, dtype=ml_dtypes.bfloat16),
    }
    n = x.shape[0] // t_core
    in_maps = [
        {"x": np.ascontiguousarray(x[i * t_core : (i + 1) * t_core]), **consts}
        for i in range(n)
    ]
    return in_maps


def _unshuffle(res_list, t_core):
    n_tiles = t_core // P
    t_full = t_core * len(res_list)
    idx = np.empty((t_full, 2), np.int32)
    wts = np.empty((t_full, 2), np.float32)
    for i, r in enumerate(res_list):
        ow = r["out_w"].reshape(P, n_tiles, 2).transpose(1, 0, 2).reshape(t_core, 2)
        oi = r["out_i"].reshape(P, n_tiles, 2).transpose(1, 0, 2).reshape(t_core, 2)
        wts[i * t_core : (i + 1) * t_core] = ow
        idx[i * t_core : (i + 1) * t_core] = oi
    return idx, wts


_NC_CACHE: dict = {}


def run(hidden_states, weight, trace=False, **kw):
    t_full = int(np.prod(np.asarray(hidden_states).shape[:-1]))
    t_core = t_full // N_CORES
    key = t_core
    if key not in _NC_CACHE:
        _NC_CACHE[key] = build_nc(t_core)
    nc = _NC_CACHE[key]
    in_maps = _prep_inputs(hidden_states, weight, t_core)
    br = run_bass_kernel_spmd(
        nc, in_maps, core_ids=list(range(len(in_maps))), trace=trace, **kw
    )
    idx, wts = _unshuffle(br.results, t_core)
    return idx, wts, br


def kernel(hidden_states, weight):
    idx, wts, _ = run(hidden_states, weight)
    return idx, wts


# revision 29
# speedup vs baseline: 1.0039x; 1.0005x over previous
"""MoE gate (router) kernel for Trainium2, 8 NeuronCores.

Computes, for hidden_states [4, 8192, 4096] fp32 and weight [64, 4096] fp32:
    logits = x @ W.T        # [T=32768, 64]
    scores = softmax(logits)
    topk_weight, topk_idx = top_k(scores, 2)
returns (topk_idx int32 [T, 2], topk_weight fp32 [T, 2]).

Sharding: tokens split evenly across 8 cores (4096 tokens/core); the small
gate weight is replicated. No collectives needed.

Per-core dataflow (hi/lo bf16 split preserves top-2 ordering; min top2/top3
logit gap in this regime is ~2e-5, so plain bf16 would mis-rank rows):
  - DMA x naturally as [128 tok, 2048 h] half-tiles.
  - PE transpose-mode flips 128x128 blocks to get h on partitions (PSUM),
    ACT cast-copies PSUM->SBUF as bf16 hi, DVE subtract produces bf16 lo.
  - PE matmul accumulates a [128, 512] PSUM tile per 512-token group over
    32 h-chunks with a PACKED stationary [Wh_c | Wl_c] (128 wide, full PE
    array): rows 0:64 accumulate xh@Wh (+ xl@Wh from a second 64-wide
    matmul into the sub-range), rows 64:128 accumulate xh@Wl. Two moving
    streams per chunk instead of three.
  - Epilogue per group: ACT copy PSUM->SBUF, PE transpose [128,128] blocks
    to token-major [128 t, 128 e2], ACT copy hi half to SBUF, DVE add gives
    logits [128, 64]; VectorE max/max_index yields top-8; ScalarE Exp with
    accum_out gives the softmax denominator; reciprocal+scale produce the
    two weights.
  - Results staged in SBUF [128, 2*n_tok_tiles], single DMA out; host
    unshuffles the [tile, partition] interleave.
"""

import sys

for _p in ("/opt/trn_rl_repo", "/root/.axon_site/_ro/trn_rl_repo"):
    if _p not in sys.path:
        sys.path.append(_p)

import numpy as np

import concourse.bass as bass
import concourse.bacc as bacc
import concourse.mybir as mybir
from concourse.tile import TileContext
from concourse.bass_utils import run_bass_kernel_spmd

N_CORES = 8
H = 4096
E = 64
P = 128
N_CHUNK = H // P  # 32 contraction chunks
TOK_GRP = 512  # tokens per PSUM logits accumulation group
F32 = mybir.dt.float32
BF16 = mybir.dt.bfloat16
I32 = mybir.dt.int32
U32 = mybir.dt.uint32


def build_nc(t_core: int) -> bass.Bass:
    """Build the per-core Bass module for t_core tokens."""
    assert t_core % TOK_GRP == 0
    n_grp = t_core // TOK_GRP
    n_tiles = t_core // P  # token tiles of 128

    # Bacc (not raw Bass): its compile() pipeline legalizes semaphore waits
    # (move_matmul_waits_to_ldweights, event-sem conversion) for the 1-wait
    # EVENTS slot walrus enforces per instruction.
    nc = bacc.Bacc(trn_type="TRN2")
    x_d = nc.dram_tensor("x", [t_core, H], F32, kind="ExternalInput")
    # whl layout: whl[p, c*128 + j] = Wh[j, 128c+p] for j<64, Wl[j-64, ...]
    # for j>=64 (host-prepared). The xl matmul reads the Wh half directly.
    whl_d = nc.dram_tensor("whl", [P, N_CHUNK * 2 * E], BF16, kind="ExternalInput")
    idb_d = nc.dram_tensor("ident_b", [8, 8], BF16, kind="ExternalInput")
    id_d = nc.dram_tensor("ident", [P, P], F32, kind="ExternalInput")
    ow_d = nc.dram_tensor("out_w", [P, 2 * n_tiles], F32, kind="ExternalOutput")
    oi_d = nc.dram_tensor("out_i", [P, 2 * n_tiles], I32, kind="ExternalOutput")

    with TileContext(nc) as tc:
        with (
            tc.tile_pool(name="const", bufs=1) as cpool,
            tc.tile_pool(name="xnat", bufs=16) as xpool,
            tc.tile_pool(name="xt", bufs=8) as xtpool,
            tc.tile_pool(name="ptr", bufs=3, space="PSUM") as pt_pool,
            tc.tile_pool(name="plog", bufs=2, space="PSUM") as pl_pool,
            tc.tile_pool(name="plt", bufs=1, space="PSUM") as plt_pool,
            tc.tile_pool(name="sac", bufs=1, space="PSUM") as sac_pool,
            tc.tile_pool(name="sacb", bufs=1, space="PSUM") as sacb_pool,
            tc.tile_pool(name="small", bufs=4) as spool,
            tc.tile_pool(name="outs", bufs=1) as opool,
        ):
            # Small consts first so the PE absorbers/warmup unblock early,
            # then the weights, then the x stream follows on the same ring.
            ident = cpool.tile([P, P], F32)
            nc.sync.dma_start(ident[:], id_d[:])
            idb_sb = cpool.tile([8, 8], BF16)
            nc.sync.dma_start(idb_sb[:], idb_d[:])
            whl_sb = cpool.tile([P, N_CHUNK * 2 * E], BF16)
            nc.sync.dma_start(whl_sb[:], whl_d[:])
            ow_sb = opool.tile([P, 2 * n_tiles], F32)
            oi_sb = opool.tile([P, 2 * n_tiles], I32)

            # Every TPB instruction has ONE sem-wait slot, and walrus cannot
            # split multi-waits for the fused fp32 matmul. So each DMA'd tile
            # gets a tiny sacrificial 8x8 PE transpose ("absorber") that
            # carries the DMA wait; real PE work then sees the tick as
            # observed and needs at most one other wait. Absorber outputs go
            # to distinct columns of one never-recycled PSUM bank (no WAW).
            # cols 0..239: HAM-warmup scratch; cols 240+: absorber outputs
            sac = sac_pool.tile([4, 240 + 2 * (8 + 8 * n_grp)], F32)
            n_sac = 0

            def absorb(src_ap):
                nonlocal n_sac
                nc.tensor.transpose(
                    sac[0:2, 240 + 2 * n_sac : 242 + 2 * n_sac],
                    src_ap,
                    ident[0:2, 0:2],
                )
                n_sac += 1

            absorb(ident[0:2, 0:2])
            # bf16 absorbers need a bf16 PSUM target (transpose output
            # dtype must match input) and a bf16 identity rhs.
            sac_bf = sacb_pool.tile([8, 24], BF16)
            nc.tensor.transpose(sac_bf[:, 0:8], idb_sb[:], idb_sb[:])
            # HAM warmup: ~6us of back-to-back matmuls while whl and the
            # first x tiles stream in, so real work starts at 2.4 GHz
            # instead of paying ~20us of half-clock ramp. The moving operand
            # is a bf16 BITCAST view of ident (values irrelevant, arrives
            # ~10us before whl), so the warmup gates on nothing but the two
            # small const DMAs. They reuse the sac bank (serial same-engine
            # WAW, no sems needed); later absorber writes order behind them.
            ident_bf = ident.bitcast(BF16)
            for _ in range(50):
                nc.tensor.matmul(
                    sac[:, 0:240], idb_sb[0:8, 0:4], ident_bf[0:8, 0:240],
                    start=True, stop=True, skip_group_check=True,
                )
            # whl's DMA-wait absorber sits AFTER the warmup so it cannot
            # delay the ramp; mm1 of chunk 0 is the first real consumer.
            nc.tensor.transpose(sac_bf[:, 8:16], whl_sb[0:8, 0:8], idb_sb[:])

            HH = H // 2

            # Software pipeline, carried ACROSS group boundaries: the matmul
            # pair for chunk (g, c) is emitted after the transposes of chunk
            # c+DELAY (possibly in group g+1), so the serial ACT-hi -> DVE-lo
            # chain always finishes before PE program order reaches mm2, and
            # the old per-group pend flush (6 back-to-back bf16 matmuls, each
            # paying the stationary-file serialization) disappears. Each
            # group's epilogue is emitted right after its stop=True matmul
            # pops from the queue.
            DELAY = 5
            pend = []  # (logits_ps, g, c, xh_sb, xl_sb)

            def emit_epilogue(g, logits_ps):
                # psum [128, 512]: rows 0:64 = xh@Wh + xl@Wh, rows 64:128 =
                # xh@Wl. Transpose each 128-token block token-major into one
                # 3D l_ps bank [P, 4, 128]; the lo/hi expert halves are then
                # combined for the WHOLE group with a single strided ACT copy
                # + a single DVE add, and e2/recip batch across blocks too.
                lt_sb = spool.tile([P, TOK_GRP], F32, tag="lt", name=f"lt_{g}")
                nc.scalar.copy(lt_sb[:], logits_ps[:])
                l_ps = plt_pool.tile([P, 4, P], F32, tag="lps", name=f"lps_{g}")
                for tb in range(4):
                    nc.tensor.transpose(
                        l_ps[:, tb, :], lt_sb[:, bass.ts(tb, P)], ident[:]
                    )
                hi_sb = spool.tile([P, 4, E], F32, tag="hi", name=f"hi_{g}")
                # ACT (not DVE) so the l_ps bank WAR release stays on the
                # ACT sem PE already tracks — keeps PE waits ≤1 per inst.
                nc.scalar.copy(hi_sb[:], l_ps[:, :, E : 2 * E])
                l_sb = spool.tile([P, 4, E], F32, tag="lsb", name=f"lsb_{g}")
                nc.vector.tensor_tensor(
                    l_sb[:], l_ps[:, :, 0:E], hi_sb[:], mybir.AluOpType.add
                )
                mx = spool.tile([P, 4, 8], F32, tag="mx", name=f"mx_{g}")
                mi = spool.tile([P, 4, 8], U32, tag="mi", name=f"mi_{g}")
                ssum = spool.tile([P, 4], F32, tag="ss", name=f"ss_{g}")
                for tb in range(4):
                    col = g * 4 + tb
                    nc.vector.max(mx[:, tb, :], l_sb[:, tb, :])
                    nc.vector.max_index(mi[:, tb, :], mx[:, tb, :], l_sb[:, tb, :])
                    ex = spool.tile([P, E], F32, tag="ex", name=f"ex_{col}")
                    nc.scalar.activation(
                        ex[:],
                        l_sb[:, tb, :],
                        mybir.ActivationFunctionType.Exp,
                        accum_out=ssum[:, tb : tb + 1],
                    )
                e2 = spool.tile([P, 4, 2], F32, tag="e2", name=f"e2_{g}")
                nc.scalar.activation(
                    e2[:], mx[:, :, 0:2], mybir.ActivationFunctionType.Exp
                )
                rec = spool.tile([P, 4], F32, tag="rc", name=f"rc_{g}")
                nc.vector.reciprocal(rec[:], ssum[:])
                for tb in range(4):
                    col = g * 4 + tb
                    nc.vector.tensor_scalar(
                        ow_sb[:, bass.ts(col, 2)],
                        e2[:, tb, :],
                        rec[:, tb : tb + 1],
                        None,
                        op0=mybir.AluOpType.mult,
                    )
                    nc.vector.tensor_copy(
                        oi_sb[:, bass.ts(col, 2)], mi[:, tb, 0:2]
                    )

            def emit_mms(logits_ps, g, c, xh_sb, xl_sb):
                nc.tensor.matmul(
                    logits_ps[:], whl_sb[:, bass.ts(c, 2 * E)], xh_sb[:],
                    start=(c == 0), stop=False, skip_group_check=True,
                )
                nc.tensor.matmul(
                    logits_ps[0:E, :],
                    whl_sb[:, c * 2 * E : c * 2 * E + E],
                    xl_sb[:],
                    start=False, stop=(c == N_CHUNK - 1),
                    skip_group_check=True,
                )
                if c == N_CHUNK - 1:
                    emit_epilogue(g, logits_ps)

            for g in range(n_grp):
                # Two half-H tiles per token block: halves the DMA granularity
                # (1 MB each) so the next group's first-half loads can start a
                # half-group earlier — removes the ~6us group-boundary stalls.
                # Emit hh=0 loads for all four blocks first: the first 16
                # chunks only need the hh=0 halves, so they arrive sooner.
                xnats = [[None, None] for _ in range(4)]
                for hh in range(2):
                    for tb in range(4):
                        row = bass.ts(g * 4 + tb, P)
                        xn = xpool.tile(
                            [P, HH], F32, tag="xn", name=f"xn_{g}_{tb}_{hh}"
                        )
                        if g == 0 and hh == 0:
                            # Group 0's first halves land as two quarter-size
                            # DMAs: region-aware dep tracking lets chunk 0's
                            # transposes start after just the first quarter,
                            # halving the cold-start tile latency.
                            HQ = HH // 2
                            nc.sync.dma_start(
                                xn[:, 0:HQ], x_d[row, 0:HQ]
                            )
                            absorb(xn[0:2, 0:2])
                            nc.sync.dma_start(
                                xn[:, HQ:HH], x_d[row, HQ:HH]
                            )
                            absorb(xn[0:2, HQ : HQ + 2])
                        else:
                            nc.sync.dma_start(
                                xn[:], x_d[row, hh * HH : (hh + 1) * HH]
                            )
                            absorb(xn[0:2, 0:2])
                        xnats[tb][hh] = xn

                logits_ps = pl_pool.tile([P, TOK_GRP], F32, tag="lg", name=f"lg_{g}")
                for c in range(N_CHUNK):
                    xt_ps = pt_pool.tile(
                        [P, TOK_GRP], F32, tag="xtp", name=f"xtp_{g}_{c}"
                    )
                    for tb in range(4):
                        nc.tensor.transpose(
                            xt_ps[:, bass.ts(tb, P)],
                            xnats[tb][c // 16][:, bass.ts(c % 16, P)],
                            ident[:],
                        )
                    xh_sb = xtpool.tile(
                        [P, TOK_GRP], BF16, tag="xh", name=f"xh_{g}_{c}"
                    )
                    xl_sb = xtpool.tile(
                        [P, TOK_GRP], BF16, tag="xl", name=f"xl_{g}_{c}"
                    )
                    nc.scalar.copy(xh_sb[:], xt_ps[:])
                    nc.vector.tensor_tensor(
                        xl_sb[:], xt_ps[:], xh_sb[:],
                        mybir.AluOpType.subtract,
                    )
                    pend.append((logits_ps, g, c, xh_sb, xl_sb))
                    if len(pend) > DELAY:
                        emit_mms(*pend.pop(0))

            while pend:
                emit_mms(*pend.pop(0))

            nc.sync.dma_start(ow_d[:], ow_sb[:])
            nc.sync.dma_start(oi_d[:], oi_sb[:])
    nc.compile()
    return nc


def _prep_inputs(hidden_states, weight, t_core):
    import ml_dtypes

    x = np.ascontiguousarray(
        np.asarray(hidden_states, dtype=np.float32).reshape(-1, H)
    )
    w = np.asarray(weight, dtype=np.float32)
    wh = w.astype(ml_dtypes.bfloat16)
    wl = (w - wh.astype(np.float32)).astype(ml_dtypes.bfloat16)
    whl = np.zeros((P, N_CHUNK * 2 * E), dtype=ml_dtypes.bfloat16)
    for c in range(N_CHUNK):
        whl[:, c * 2 * E : c * 2 * E + E] = wh[:, c * P : (c + 1) * P].T
        whl[:, c * 2 * E + E : (c + 1) * 2 * E] = wl[:, c * P : (c + 1) * P].T
    consts = {
        "whl": whl,
        "ident": np.eye(P, dtype=np.float32),
        "ident_b": np.eye(8, dtype=ml_dtypes.bfloat16),
    }
    n = x.shape[0] // t_core
    in_maps = [
        {"x": np.ascontiguousarray(x[i * t_core : (i + 1) * t_core]), **consts}
        for i in range(n)
    ]
    return in_maps


def _unshuffle(res_list, t_core):
    n_tiles = t_core // P
    t_full = t_core * len(res_list)
    idx = np.empty((t_full, 2), np.int32)
    wts = np.empty((t_full, 2), np.float32)
    for i, r in enumerate(res_list):
        ow = r["out_w"].reshape(P, n_tiles, 2).transpose(1, 0, 2).reshape(t_core, 2)
        oi = r["out_i"].reshape(P, n_tiles, 2).transpose(1, 0, 2).reshape(t_core, 2)
        wts[i * t_core : (i + 1) * t_core] = ow
        idx[i * t_core : (i + 1) * t_core] = oi
    return idx, wts


_NC_CACHE: dict = {}


def run(hidden_states, weight, trace=False, **kw):
    t_full = int(np.prod(np.asarray(hidden_states).shape[:-1]))
    t_core = t_full // N_CORES
    key = t_core
    if key not in _NC_CACHE:
        _NC_CACHE[key] = build_nc(t_core)
    nc = _NC_CACHE[key]
    in_maps = _prep_inputs(hidden_states, weight, t_core)
    br = run_bass_kernel_spmd(
        nc, in_maps, core_ids=list(range(len(in_maps))), trace=trace, **kw
    )
    idx, wts = _unshuffle(br.results, t_core)
    return idx, wts, br


def kernel(hidden_states, weight):
    idx, wts, _ = run(hidden_states, weight)
    return idx, wts
